# revision 27
# baseline (speedup 1.0000x reference)
"""CAM (channel attention) module kernel for Trainium2, 8-core data-parallel.

Reference computation (per sample b):
    q = conv2d(x, Wq, stride2, 2x2) -> [C, 4096]
    k = conv2d(x, Wk, stride2, 2x2) -> [C, 4096]
    v = conv2d(x, Wv, 1x1)          -> [C, 16384]
    E = q @ k^T                      [C, C]
    att = softmax(rowmax(E) - E)   (== softmin over rows)
    out = att @ v -> [C, H, W]

Kernel strategy (one sample per NeuronCore, B=8 over 8 cores):
  - The softmax is extremely peaked (energy entries span +-200), so energy
    errors are amplified exponentially: q/k need ~16+ mantissa bits. The
    convs use split-bf16 (x = xh + xl, W = Wh + Wl, conv = Wh@xh + Wh@xl +
    Wl@xh): 3 full-rate bf16 passes.
  - q/k land in PSUM [c, n], are evacuated to SBUF f32 and PE-transposed
    (fp32, via identity) to [n, c] chunks for the energy contraction.
    (A DMA-XBAR transpose variant was tried: ~6% faster but intermittently
    read stale data on first execution despite statically-verified
    semaphore coverage -- PE transposes are the reliable path.)
  - energy: E accumulated in one PSUM bank from 4 fp32 chunk matmuls per
    band, interleaved one band behind the conv stream so the PE never
    idles.
  - softmax via one DVE row-min + one ScalarE exp (bias=rowmin, scale=-1)
    with fused row-sum; M^T = Wv^T att^T on PE.
  - out = M @ xh computed in a single bf16 pass (Mb is the one stationary
    for all 32 output matmuls; xh = bf16(x) is already resident from the
    conv phase). Output is stored to HBM as fp16 (halves the store bytes;
    |out| <= ~6 so fp16 range is safe) and widened to f32 on the host.
    Simulated end-to-end rel err of this scheme: ~2.5e-3 (gate is 2e-2).
  - startup: x band 0 is DMA'd first at full bandwidth, then the conv
    weights, then the remaining bands; ~30 throwaway bf16 matmuls warm the
    PE HAM clock gate (1.2 -> 2.4 GHz) while the first band loads.
"""

import numpy as np

B, C, H, W = 8, 128, 128, 128
HW = H * W           # 16384
N_CORES = 8
NB = 8               # number of H-bands (16 input rows each)
BAND = HW // NB      # 2048 x columns per band
QN = (H // 2) * (W // 2)  # 4096 conv output positions
QCHUNK = QN // NB    # 512 conv outputs per band

_CACHE = {}


def _build_program(with_qk_bias: bool, with_v_bias: bool):
    import concourse.tile as tile
    from concourse import bacc, mybir
    from concourse.masks import make_identity

    f32 = mybir.dt.float32
    bf16 = mybir.dt.bfloat16
    f16 = mybir.dt.float16
    Ident = mybir.ActivationFunctionType.Identity
    CopyF = mybir.ActivationFunctionType.Copy
    nc = bacc.Bacc(
        "TRN2", target_bir_lowering=False, debug=False, num_devices=N_CORES)

    x_d = nc.declare_dram_parameter("x", [C, HW], f32, isOutput=False)
    wqk_d = nc.declare_dram_parameter("wqk", [C, 8 * C], f32, isOutput=False)
    wv_d = nc.declare_dram_parameter("wv", [C, C], f32, isOutput=False)
    if with_qk_bias:
        bq_d = nc.declare_dram_parameter("bq", [C, 1], f32, isOutput=False)
        bk_d = nc.declare_dram_parameter("bk", [C, 1], f32, isOutput=False)
    if with_v_bias:
        bv_d = nc.declare_dram_parameter("bv", [C, 1], f32, isOutput=False)
    out_d = nc.declare_dram_parameter("out", [C, HW], f16, isOutput=True)

    with tile.TileContext(nc) as tc:
        with (
            tc.tile_pool(name="const", bufs=1) as const,
            tc.tile_pool(name="xstage", bufs=3) as xstage,
            tc.tile_pool(name="xhp", bufs=1) as xhp,
            tc.tile_pool(name="xlp", bufs=3) as xlp,
            tc.tile_pool(name="qkstage", bufs=4) as qkstage,
            tc.tile_pool(name="qkT", bufs=6) as qkT,
            tc.tile_pool(name="oout", bufs=3) as oout,
            tc.tile_pool(name="small", bufs=2) as small,
            tc.tile_pool(name="pacc", bufs=4, space="PSUM") as pacc,
            tc.tile_pool(name="ptp", bufs=2, space="PSUM") as ptp,
            tc.tile_pool(name="psm", bufs=1, space="PSUM") as psm,
        ):
            # ---- input DMAs, consumption order, all on the sync queue so
            # each transfer gets the full per-core HBM bandwidth ----
            x_sb = []
            x0 = xstage.tile([C, BAND], f32, tag="x")
            nc.sync.dma_start(out=x0, in_=x_d[:, 0:BAND])
            x_sb.append(x0)
            wqk_sb = const.tile([C, 8 * C], f32, tag="wqk")
            nc.sync.dma_start(out=wqk_sb, in_=wqk_d[:, :])
            wqT_sb = wqk_sb[:, 0:4 * C]
            wkT_sb = wqk_sb[:, 4 * C:8 * C]
            if with_qk_bias:
                bq_sb = const.tile([C, 1], f32, tag="bq")
                nc.sync.dma_start(out=bq_sb, in_=bq_d[:, :])
                bk_sb = const.tile([C, 1], f32, tag="bk")
                nc.sync.dma_start(out=bk_sb, in_=bk_d[:, :])
            for j in range(1, NB):
                t = xstage.tile([C, BAND], f32, tag="x", name=f"x{j}")
                nc.sync.dma_start(out=t, in_=x_d[:, j * BAND:(j + 1) * BAND])
                x_sb.append(t)
            wv_sb = const.tile([C, C], f32, tag="wv")
            nc.sync.dma_start(out=wv_sb, in_=wv_d[:, :])
            if with_v_bias:
                bv_sb = const.tile([C, 1], f32, tag="bv")
                nc.sync.dma_start(out=bv_sb, in_=bv_d[:, :])

            ident = const.tile([128, 128], f32, tag="ident")
            make_identity(nc, ident)
            identb = const.tile([128, 128], bf16, tag="identb")
            nc.vector.tensor_copy(identb, ident)

            # ---- PE pre-warm: throwaway bf16 matmuls (~100ns cold each)
            # keep the PE busy from ~6.5us so the HAM clock gate has
            # un-throttled (2.4 GHz) by the time the first conv runs ----
            for dw in range(44):
                scr = pacc.tile([128, 128], f32, tag="acc", name=f"warm{dw}")
                nc.tensor.matmul(scr, lhsT=identb, rhs=identb,
                                 start=True, stop=True)

            # split conv weights into bf16 hi/lo (Scalar + GpSimd; DVE is
            # busy with the band-0 x split)
            def split_w(w_f32, name):
                hi = const.tile([C, 4 * C], bf16, tag=f"{name}hi")
                nc.scalar.activation(out=hi, in_=w_f32, func=CopyF,
                                     bias=0.0, scale=1.0)
                lo = const.tile([C, 4 * C], bf16, tag=f"{name}lo")
                nc.gpsimd.tensor_tensor(
                    out=lo, in0=w_f32, in1=hi,
                    op=mybir.AluOpType.subtract)
                return hi, lo

            wqh, wql = split_w(wqT_sb, "wq")
            wkh, wkl = split_w(wkT_sb, "wk")

            # xh stays resident for all bands (reused by the output pass)
            xh_sb = [xhp.tile([C, BAND], bf16, tag=f"xh{j}", name=f"xh{j}")
                     for j in range(NB)]

            def conv_band(j, wh, wl, xr_v, xl_v, xh_first=False):
                """12 accumulating matmuls -> PSUM [128, 512]. With
                xh_first, the 8 xh-only passes run before the 4 xl passes
                (band 0: xl is still being computed when xh lands)."""
                acc = pacc.tile([128, QCHUNK], f32, tag="acc")
                mms = []
                for ab in range(4):
                    a, bb = ab // 2, ab % 2
                    mms.append((0, wh[:, ab * C:(ab + 1) * C],
                                xr_v[:, :, a, :, bb]))
                    mms.append((1, wh[:, ab * C:(ab + 1) * C],
                                xl_v[:, :, a, :, bb]))
                    mms.append((0, wl[:, ab * C:(ab + 1) * C],
                                xr_v[:, :, a, :, bb]))
                if xh_first:
                    mms.sort(key=lambda t: t[0])
                for n_mm, (_, lhsT, rhs) in enumerate(mms):
                    nc.tensor.matmul(acc, lhsT=lhsT, rhs=rhs,
                                     start=(n_mm == 0), stop=(n_mm == 11))
                return acc

            # energy accumulator lives across the whole conv phase
            E = psm.tile([128, 128], f32, tag="E")
            e_idx = [0]
            qkT_tiles = {}

            qc_tiles = {}

            def evac_qk(j, acc, which, bias_sb):
                """PSUM [c,512] f32 -> SBUF f32 (DVE / Scalar-with-bias)."""
                qc = qkstage.tile([128, QCHUNK], f32, tag="qc",
                                  name=f"{which}c{j}")
                if bias_sb is not None:
                    nc.scalar.activation(out=qc, in_=acc, func=Ident,
                                         bias=bias_sb[:, 0:1], scale=1.0)
                else:
                    nc.vector.tensor_copy(qc, acc)
                qc_tiles[(which, j)] = qc

            def emit_transposes(j):
                """4 PE transposes per tensor -> PSUM -> SBUF chunk tile."""
                for which in ("q", "k"):
                    qc = qc_tiles.pop((which, j))
                    tp = ptp.tile([128, QCHUNK], f32, tag="tp")
                    for t in range(4):
                        nc.tensor.transpose(
                            tp[:, t * 128:(t + 1) * 128],
                            qc[:, t * 128:(t + 1) * 128], ident)
                    T = qkT.tile([128, QCHUNK], f32, tag="qkT",
                                 name=f"{which}T{j}")
                    nc.scalar.activation(out=T, in_=tp, func=CopyF,
                                         bias=0.0, scale=1.0)
                    qkT_tiles[(which, j)] = T

            def emit_energy(j):
                qT = qkT_tiles.pop(("q", j))
                kT = qkT_tiles.pop(("k", j))
                for t in range(4):
                    nc.tensor.matmul(
                        E,
                        lhsT=qT[:, t * 128:(t + 1) * 128],
                        rhs=kT[:, t * 128:(t + 1) * 128],
                        start=(e_idx[0] == 0),
                        stop=(e_idx[0] == NB * 4 - 1))
                    e_idx[0] += 1

            # split x into bf16 hi (DVE) / lo (GpSimd; its tensor_tensor
            # is full speed -- only its dtype CASTs are slow). Emitted one
            # band AHEAD of the conv loop so the next band's split is
            # never queued behind this band's PSUM evacuations on DVE.
            xl_sb = {}

            def split_band(j):
                xl_t = xlp.tile([C, BAND], bf16, tag="xl", name=f"xl{j}")
                nc.vector.tensor_copy(xh_sb[j], x_sb[j])
                nc.gpsimd.tensor_tensor(
                    out=xl_t, in0=x_sb[j], in1=xh_sb[j],
                    op=mybir.AluOpType.subtract)
                xl_sb[j] = xl_t

            split_band(0)
            for j in range(NB):
                if j + 1 < NB:
                    split_band(j + 1)
                xh_t, xl_t = xh_sb[j], xl_sb.pop(j)
                xr_v = xh_t[:].rearrange(
                    "p (i a w b) -> p i a w b", i=8, a=2, w=64, b=2)
                xl_v = xl_t[:].rearrange(
                    "p (i a w b) -> p i a w b", i=8, a=2, w=64, b=2)
                acc_q = conv_band(j, wqh, wql, xr_v, xl_v, xh_first=(j == 0))
                acc_k = conv_band(j, wkh, wkl, xr_v, xl_v, xh_first=(j == 0))
                evac_qk(j, acc_q, "q", bq_sb if with_qk_bias else None)
                evac_qk(j, acc_k, "k", bk_sb if with_qk_bias else None)
                # transposes one band behind (their evac needs the conv
                # window), energy two behind (needs the transpose evac)
                if j >= 1:
                    emit_transposes(j - 1)
                if j >= 2:
                    emit_energy(j - 2)
            emit_transposes(NB - 1)
            emit_energy(NB - 2)
            # cover the last transpose-evac latency with throwaway matmuls
            for dw in range(8):
                scr = pacc.tile([128, 128], f32, tag="acc",
                                name=f"wpre{dw}")
                nc.tensor.matmul(scr, lhsT=identb, rhs=identb,
                                 start=True, stop=True)
            emit_energy(NB - 1)
            for dw in range(22):
                scr = pacc.tile([128, 128], f32, tag="acc",
                                name=f"wsm{dw}")
                nc.tensor.matmul(scr, lhsT=identb, rhs=identb,
                                 start=True, stop=True)

            # softmin over rows: att = exp(rowmin - E) / Z
            mmin = small.tile([128, 1], f32, tag="mmin")
            nc.vector.tensor_reduce(
                out=mmin, in_=E, axis=mybir.AxisListType.X,
                op=mybir.AluOpType.min)
            w_sb = small.tile([128, 128], f32, tag="w")
            zsum = small.tile([128, 1], f32, tag="z")
            nc.scalar.activation(
                out=w_sb, in_=E, func=mybir.ActivationFunctionType.Exp,
                bias=mmin[:, 0:1], scale=-1.0, accum_out=zsum[:, 0:1])
            rz = small.tile([128, 1], f32, tag="rz")
            nc.vector.reciprocal(rz, zsum)
            # normalize here (cheap [128,128] op) so the per-band output
            # evacuations are plain casts instead of per-element scales
            att = small.tile([128, 128], f32, tag="att")
            nc.vector.tensor_scalar_mul(att, w_sb, rz[:, 0:1])

            attT_p = psm.tile([128, 128], f32, tag="s2")
            nc.tensor.transpose(attT_p, att, ident)
            attT = small.tile([128, 128], f32, tag="attT")
            nc.vector.tensor_copy(attT, attT_p)

            # M^T[c2, c] = sum_d Wv[d, c2] attT[d, c]
            MT_p = psm.tile([128, 128], f32, tag="s2")
            nc.tensor.matmul(MT_p, lhsT=wv_sb, rhs=attT, start=True, stop=True)
            Mb = small.tile([128, 128], bf16, tag="Mb")
            nc.vector.tensor_copy(Mb, MT_p)

            if with_v_bias:
                abv_p = psm.tile([128, 1], f32, tag="s2")
                nc.tensor.matmul(abv_p, lhsT=attT, rhs=bv_sb[:, 0:1],
                                 start=True, stop=True)
                abv = small.tile([128, 1], f32, tag="abv")
                nc.vector.tensor_copy(abv, abv_p)

            # out[c, n] = sum_c2 M[c, c2] xh[c2, n] in one bf16 pass; Mb is
            # the single stationary for all 32 matmuls. Evacuate to fp16 and
            # store fp16 (widened to f32 on the host). Stores only on the
            # two hardware-DGE queues (sync, scalar) -- gpsimd's software
    	    # DGE is far slower.
            out_dma_engines = [nc.sync, nc.scalar]
            for j in range(NB):
                o_band = oout.tile([128, BAND], f16, tag="oband")
                o_ps = [pacc.tile([128, 512], f32, tag="acc",
                                  name=f"ops{j}_{s}")
                        for s in range(4)]
                for s in range(4):
                    nc.tensor.matmul(
                        o_ps[s], lhsT=Mb,
                        rhs=xh_sb[j][:, s * 512:(s + 1) * 512],
                        start=True, stop=True)
                for s in range(4):
                    dst = o_band[:, s * 512:(s + 1) * 512]
                    if with_v_bias:
                        nc.scalar.activation(
                            out=dst, in_=o_ps[s], func=Ident,
                            bias=abv[:, 0:1], scale=1.0)
                    elif s % 2 == 0:
                        nc.vector.tensor_copy(dst, o_ps[s])
                    else:
                        nc.scalar.activation(out=dst, in_=o_ps[s], func=CopyF,
                                             bias=0.0, scale=1.0)
                out_dma_engines[j % 2].dma_start(
                    out=out_d[:, j * BAND:(j + 1) * BAND],
                    in_=o_band[:])

    nc.compile()
    return nc


def kernel(x, Wq, bq, Wk, bk, Wv, bv):
    from concourse.bass_utils import run_bass_kernel_spmd

    x = np.ascontiguousarray(np.asarray(x, dtype=np.float32))
    Wq = np.asarray(Wq, dtype=np.float32)
    Wk = np.asarray(Wk, dtype=np.float32)
    Wv = np.asarray(Wv, dtype=np.float32)
    bq = np.asarray(bq, dtype=np.float32)
    bk = np.asarray(bk, dtype=np.float32)
    bv = np.asarray(bv, dtype=np.float32)

    with_qk_bias = bool(np.any(bq) or np.any(bk))
    with_v_bias = bool(np.any(bv))

    key = (with_qk_bias, with_v_bias)
    if key not in _CACHE:
        _CACHE[key] = _build_program(with_qk_bias, with_v_bias)
    nc = _CACHE[key]

    # weight layout prep: wT[cin, ab*128 + c] = W[c, cin, a, b];
    # q and k weights packed into one tensor for a single early DMA
    wqT = Wq.transpose(1, 2, 3, 0).reshape(C, 4 * C)
    wkT = Wk.transpose(1, 2, 3, 0).reshape(C, 4 * C)
    wqk = np.ascontiguousarray(np.concatenate([wqT, wkT], axis=1))
    wv = np.ascontiguousarray(Wv.reshape(C, C))

    in_maps = []
    for b in range(B):
        m = {
            "x": np.ascontiguousarray(x[b].reshape(C, HW)),
            "wqk": wqk,
            "wv": wv,
        }
        if with_qk_bias:
            m["bq"] = np.ascontiguousarray(bq.reshape(C, 1))
            m["bk"] = np.ascontiguousarray(bk.reshape(C, 1))
        if with_v_bias:
            m["bv"] = np.ascontiguousarray(bv.reshape(C, 1))
        in_maps.append(m)

    res = run_bass_kernel_spmd(nc, in_maps, list(range(N_CORES)))
    out = np.stack([np.asarray(res.results[i]["out"], dtype=np.float32)
                    for i in range(N_CORES)])
    return out.reshape(B, C, H, W)


# revision 29
# speedup vs baseline: 1.0237x; 1.0237x over previous
"""CAM (channel attention) module kernel for Trainium2, 8-core data-parallel.

Reference computation (per sample b):
    q = conv2d(x, Wq, stride2, 2x2) -> [C, 4096]
    k = conv2d(x, Wk, stride2, 2x2) -> [C, 4096]
    v = conv2d(x, Wv, 1x1)          -> [C, 16384]
    E = q @ k^T                      [C, C]
    att = softmax(rowmax(E) - E)   (== softmin over rows)
    out = att @ v -> [C, H, W]

Kernel strategy (one sample per NeuronCore, B=8 over 8 cores):
  - The softmax is extremely peaked (energy entries span +-200), so energy
    errors are amplified exponentially: q/k need ~16+ mantissa bits. The
    convs use split-bf16 (x = xh + xl, W = Wh + Wl, conv = Wh@xh + Wh@xl +
    Wl@xh): 3 full-rate bf16 passes.
  - q/k land in PSUM [c, n], are evacuated to SBUF f32 and PE-transposed
    (fp32, via identity) to [n, c] chunks for the energy contraction.
    (A DMA-XBAR transpose variant was tried: ~6% faster but intermittently
    read stale data on first execution despite statically-verified
    semaphore coverage -- PE transposes are the reliable path.)
  - energy: E accumulated in one PSUM bank from 4 fp32 chunk matmuls per
    band, interleaved one band behind the conv stream so the PE never
    idles.
  - softmax via one DVE row-min + one ScalarE exp (bias=rowmin, scale=-1)
    with fused row-sum; M^T = Wv^T att^T on PE.
  - out = M @ xh computed in a single bf16 pass (Mb is the one stationary
    for all 32 output matmuls; xh = bf16(x) is already resident from the
    conv phase). Output is stored to HBM as fp16 (halves the store bytes;
    |out| <= ~6 so fp16 range is safe) and widened to f32 on the host.
    Simulated end-to-end rel err of this scheme: ~2.5e-3 (gate is 2e-2).
  - startup: x band 0 is DMA'd first at full bandwidth, then the conv
    weights, then the remaining bands; ~30 throwaway bf16 matmuls warm the
    PE HAM clock gate (1.2 -> 2.4 GHz) while the first band loads.
"""

import numpy as np

B, C, H, W = 8, 128, 128, 128
HW = H * W           # 16384
N_CORES = 8
NB = 8               # number of H-bands (16 input rows each)
BAND = HW // NB      # 2048 x columns per band
QN = (H // 2) * (W // 2)  # 4096 conv output positions
QCHUNK = QN // NB    # 512 conv outputs per band

_CACHE = {}


def _build_program(with_qk_bias: bool, with_v_bias: bool):
    import concourse.tile as tile
    from concourse import bacc, mybir
    from concourse.masks import make_identity

    f32 = mybir.dt.float32
    bf16 = mybir.dt.bfloat16
    f16 = mybir.dt.float16
    Ident = mybir.ActivationFunctionType.Identity
    CopyF = mybir.ActivationFunctionType.Copy
    nc = bacc.Bacc(
        "TRN2", target_bir_lowering=False, debug=False, num_devices=N_CORES)

    x_d = nc.declare_dram_parameter("x", [C, HW], f32, isOutput=False)
    wqk_d = nc.declare_dram_parameter("wqk", [C, 8 * C], f32, isOutput=False)
    wv_d = nc.declare_dram_parameter("wv", [C, C], f32, isOutput=False)
    if with_qk_bias:
        bq_d = nc.declare_dram_parameter("bq", [C, 1], f32, isOutput=False)
        bk_d = nc.declare_dram_parameter("bk", [C, 1], f32, isOutput=False)
    if with_v_bias:
        bv_d = nc.declare_dram_parameter("bv", [C, 1], f32, isOutput=False)
    out_d = nc.declare_dram_parameter("out", [C, HW], f16, isOutput=True)

    with tile.TileContext(nc) as tc:
        with (
            tc.tile_pool(name="const", bufs=1) as const,
            tc.tile_pool(name="xstage", bufs=3) as xstage,
            tc.tile_pool(name="xhp", bufs=1) as xhp,
            tc.tile_pool(name="xlp", bufs=3) as xlp,
            tc.tile_pool(name="qkstage", bufs=4) as qkstage,
            tc.tile_pool(name="qkT", bufs=6) as qkT,
            tc.tile_pool(name="oout", bufs=3) as oout,
            tc.tile_pool(name="small", bufs=2) as small,
            tc.tile_pool(name="pacc", bufs=4, space="PSUM") as pacc,
            tc.tile_pool(name="ptp", bufs=2, space="PSUM") as ptp,
            tc.tile_pool(name="psm", bufs=1, space="PSUM") as psm,
        ):
            # ---- input DMAs, consumption order, all on the sync queue so
            # each transfer gets the full per-core HBM bandwidth ----
            x_sb = []
            x0 = xstage.tile([C, BAND], f32, tag="x")
            nc.sync.dma_start(out=x0, in_=x_d[:, 0:BAND])
            x_sb.append(x0)
            wqk_sb = const.tile([C, 8 * C], f32, tag="wqk")
            nc.sync.dma_start(out=wqk_sb, in_=wqk_d[:, :])
            wqT_sb = wqk_sb[:, 0:4 * C]
            wkT_sb = wqk_sb[:, 4 * C:8 * C]
            if with_qk_bias:
                bq_sb = const.tile([C, 1], f32, tag="bq")
                nc.sync.dma_start(out=bq_sb, in_=bq_d[:, :])
                bk_sb = const.tile([C, 1], f32, tag="bk")
                nc.sync.dma_start(out=bk_sb, in_=bk_d[:, :])
            for j in range(1, NB):
                t = xstage.tile([C, BAND], f32, tag="x", name=f"x{j}")
                nc.sync.dma_start(out=t, in_=x_d[:, j * BAND:(j + 1) * BAND])
                x_sb.append(t)
            wv_sb = const.tile([C, C], f32, tag="wv")
            nc.sync.dma_start(out=wv_sb, in_=wv_d[:, :])
            if with_v_bias:
                bv_sb = const.tile([C, 1], f32, tag="bv")
                nc.sync.dma_start(out=bv_sb, in_=bv_d[:, :])

            ident = const.tile([128, 128], f32, tag="ident")
            make_identity(nc, ident)
            identb = const.tile([128, 128], bf16, tag="identb")
            nc.vector.tensor_copy(identb, ident)

            # ---- PE pre-warm: throwaway bf16 matmuls (~100ns cold each)
            # keep the PE busy from ~6.5us so the HAM clock gate has
            # un-throttled (2.4 GHz) by the time the first conv runs ----
            for dw in range(44):
                scr = pacc.tile([128, 128], f32, tag="acc", name=f"warm{dw}")
                nc.tensor.matmul(scr, lhsT=identb, rhs=identb,
                                 start=True, stop=True)

            # split conv weights into bf16 hi/lo (Scalar + GpSimd; DVE is
            # busy with the band-0 x split)
            def split_w(w_f32, name):
                hi = const.tile([C, 4 * C], bf16, tag=f"{name}hi")
                nc.scalar.activation(out=hi, in_=w_f32, func=CopyF,
                                     bias=0.0, scale=1.0)
                lo = const.tile([C, 4 * C], bf16, tag=f"{name}lo")
                nc.gpsimd.tensor_tensor(
                    out=lo, in0=w_f32, in1=hi,
                    op=mybir.AluOpType.subtract)
                return hi, lo

            wqh, wql = split_w(wqT_sb, "wq")
            wkh, wkl = split_w(wkT_sb, "wk")

            # xh stays resident for all bands (reused by the output pass)
            xh_sb = [xhp.tile([C, BAND], bf16, tag=f"xh{j}", name=f"xh{j}")
                     for j in range(NB)]

            def conv_band(j, wh, wl, xr_v, xl_v, xh_first=False):
                """12 accumulating matmuls -> PSUM [128, 512]. With
                xh_first, the 8 xh-only passes run before the 4 xl passes
                (band 0: xl is still being computed when xh lands)."""
                acc = pacc.tile([128, QCHUNK], f32, tag="acc")
                mms = []
                for ab in range(4):
                    a, bb = ab // 2, ab % 2
                    mms.append((0, wh[:, ab * C:(ab + 1) * C],
                                xr_v[:, :, a, :, bb]))
                    mms.append((1, wh[:, ab * C:(ab + 1) * C],
                                xl_v[:, :, a, :, bb]))
                    mms.append((0, wl[:, ab * C:(ab + 1) * C],
                                xr_v[:, :, a, :, bb]))
                if xh_first:
                    mms.sort(key=lambda t: t[0])
                for n_mm, (_, lhsT, rhs) in enumerate(mms):
                    nc.tensor.matmul(acc, lhsT=lhsT, rhs=rhs,
                                     start=(n_mm == 0), stop=(n_mm == 11))
                return acc

            # energy accumulator lives across the whole conv phase
            E = psm.tile([128, 128], f32, tag="E")
            e_idx = [0]
            qkT_tiles = {}

            qc_tiles = {}

            def evac_qk(j, acc, which, bias_sb):
                """PSUM [c,512] f32 -> SBUF f32 (DVE / Scalar-with-bias)."""
                qc = qkstage.tile([128, QCHUNK], f32, tag="qc",
                                  name=f"{which}c{j}")
                if bias_sb is not None:
                    nc.scalar.activation(out=qc, in_=acc, func=Ident,
                                         bias=bias_sb[:, 0:1], scale=1.0)
                else:
                    nc.vector.tensor_copy(qc, acc)
                qc_tiles[(which, j)] = qc

            def emit_transposes(j):
                """4 PE transposes per tensor -> PSUM -> SBUF chunk tile."""
                for which in ("q", "k"):
                    qc = qc_tiles.pop((which, j))
                    tp = ptp.tile([128, QCHUNK], f32, tag="tp")
                    for t in range(4):
                        nc.tensor.transpose(
                            tp[:, t * 128:(t + 1) * 128],
                            qc[:, t * 128:(t + 1) * 128], ident)
                    T = qkT.tile([128, QCHUNK], f32, tag="qkT",
                                 name=f"{which}T{j}")
                    nc.scalar.activation(out=T, in_=tp, func=CopyF,
                                         bias=0.0, scale=1.0)
                    qkT_tiles[(which, j)] = T

            def emit_energy(j):
                qT = qkT_tiles.pop(("q", j))
                kT = qkT_tiles.pop(("k", j))
                for t in range(4):
                    nc.tensor.matmul(
                        E,
                        lhsT=qT[:, t * 128:(t + 1) * 128],
                        rhs=kT[:, t * 128:(t + 1) * 128],
                        start=(e_idx[0] == 0),
                        stop=(e_idx[0] == NB * 4 - 1))
                    e_idx[0] += 1

            # split x into bf16 hi (DVE) / lo (GpSimd; its tensor_tensor
            # is full speed -- only its dtype CASTs are slow). Emitted one
            # band AHEAD of the conv loop so the next band's split is
            # never queued behind this band's PSUM evacuations on DVE.
            xl_sb = {}

            def split_band(j):
                xl_t = xlp.tile([C, BAND], bf16, tag="xl", name=f"xl{j}")
                nc.vector.tensor_copy(xh_sb[j], x_sb[j])
                nc.gpsimd.tensor_tensor(
                    out=xl_t, in0=x_sb[j], in1=xh_sb[j],
                    op=mybir.AluOpType.subtract)
                xl_sb[j] = xl_t

            split_band(0)
            for j in range(NB):
                if j + 1 < NB:
                    split_band(j + 1)
                xh_t, xl_t = xh_sb[j], xl_sb.pop(j)
                xr_v = xh_t[:].rearrange(
                    "p (i a w b) -> p i a w b", i=8, a=2, w=64, b=2)
                xl_v = xl_t[:].rearrange(
                    "p (i a w b) -> p i a w b", i=8, a=2, w=64, b=2)
                acc_q = conv_band(j, wqh, wql, xr_v, xl_v, xh_first=(j == 0))
                acc_k = conv_band(j, wkh, wkl, xr_v, xl_v, xh_first=(j == 0))
                evac_qk(j, acc_q, "q", bq_sb if with_qk_bias else None)
                evac_qk(j, acc_k, "k", bk_sb if with_qk_bias else None)
                # transposes one band behind (their evac needs the conv
                # window), energy two behind (needs the transpose evac)
                if j >= 1:
                    emit_transposes(j - 1)
                if j >= 2:
                    emit_energy(j - 2)
            emit_transposes(NB - 1)
            emit_energy(NB - 2)
            # cover the last transpose-evac latency with throwaway matmuls
            for dw in range(8):
                scr = pacc.tile([128, 128], f32, tag="acc",
                                name=f"wpre{dw}")
                nc.tensor.matmul(scr, lhsT=identb, rhs=identb,
                                 start=True, stop=True)
            emit_energy(NB - 1)
            for dw in range(22):
                scr = pacc.tile([128, 128], f32, tag="acc",
                                name=f"wsm{dw}")
                nc.tensor.matmul(scr, lhsT=identb, rhs=identb,
                                 start=True, stop=True)

            # softmin over rows: att = exp(rowmin - E) / Z
            mmin = small.tile([128, 1], f32, tag="mmin")
            nc.vector.tensor_reduce(
                out=mmin, in_=E, axis=mybir.AxisListType.X,
                op=mybir.AluOpType.min)
            w_sb = small.tile([128, 128], f32, tag="w")
            zsum = small.tile([128, 1], f32, tag="z")
            nc.scalar.activation(
                out=w_sb, in_=E, func=mybir.ActivationFunctionType.Exp,
                bias=mmin[:, 0:1], scale=-1.0, accum_out=zsum[:, 0:1])
            rz = small.tile([128, 1], f32, tag="rz")
            nc.vector.reciprocal(rz, zsum)
            # normalize here (cheap [128,128] op) so the per-band output
            # evacuations are plain casts instead of per-element scales
            att = small.tile([128, 128], f32, tag="att")
            nc.vector.tensor_scalar_mul(att, w_sb, rz[:, 0:1])

            attT_p = psm.tile([128, 128], f32, tag="s2")
            nc.tensor.transpose(attT_p, att, ident)
            attT = small.tile([128, 128], f32, tag="attT")
            nc.vector.tensor_copy(attT, attT_p)

            # M^T[c2, c] = sum_d Wv[d, c2] attT[d, c]
            MT_p = psm.tile([128, 128], f32, tag="s2")
            nc.tensor.matmul(MT_p, lhsT=wv_sb, rhs=attT, start=True, stop=True)
            Mb = small.tile([128, 128], bf16, tag="Mb")
            nc.vector.tensor_copy(Mb, MT_p)

            if with_v_bias:
                abv_p = psm.tile([128, 1], f32, tag="s2")
                nc.tensor.matmul(abv_p, lhsT=attT, rhs=bv_sb[:, 0:1],
                                 start=True, stop=True)
                abv = small.tile([128, 1], f32, tag="abv")
                nc.vector.tensor_copy(abv, abv_p)

            # out[c, n] = sum_c2 M[c, c2] xh[c2, n] in one bf16 pass; Mb is
            # the single stationary for all 32 matmuls. Evacuate to fp16 and
            # store fp16 (widened to f32 on the host). Stores only on the
            # two hardware-DGE queues (sync, scalar) -- gpsimd's software
    	    # DGE is far slower.
            out_dma_engines = [nc.sync, nc.scalar]
            for j in range(NB):
                o_band = oout.tile([128, BAND], f16, tag="oband")
                o_ps = [pacc.tile([128, 512], f32, tag="acc",
                                  name=f"ops{j}_{s}")
                        for s in range(4)]
                for s in range(4):
                    nc.tensor.matmul(
                        o_ps[s], lhsT=Mb,
                        rhs=xh_sb[j][:, s * 512:(s + 1) * 512],
                        start=True, stop=True)
                for s in range(4):
                    dst = o_band[:, s * 512:(s + 1) * 512]
                    if with_v_bias:
                        nc.scalar.activation(
                            out=dst, in_=o_ps[s], func=Ident,
                            bias=abv[:, 0:1], scale=1.0)
                    elif s % 2 == 0:
                        nc.vector.tensor_copy(dst, o_ps[s])
                    else:
                        nc.scalar.activation(out=dst, in_=o_ps[s], func=CopyF,
                                             bias=0.0, scale=1.0)
                out_dma_engines[j % 2].dma_start(
                    out=out_d[:, j * BAND:(j + 1) * BAND],
                    in_=o_band[:])

    nc.compile()
    return nc


def kernel(x, Wq, bq, Wk, bk, Wv, bv):
    from concourse.bass_utils import run_bass_kernel_spmd

    x = np.ascontiguousarray(np.asarray(x, dtype=np.float32))
    Wq = np.asarray(Wq, dtype=np.float32)
    Wk = np.asarray(Wk, dtype=np.float32)
    Wv = np.asarray(Wv, dtype=np.float32)
    bq = np.asarray(bq, dtype=np.float32)
    bk = np.asarray(bk, dtype=np.float32)
    bv = np.asarray(bv, dtype=np.float32)

    with_qk_bias = bool(np.any(bq) or np.any(bk))
    with_v_bias = bool(np.any(bv))

    key = (with_qk_bias, with_v_bias)
    if key not in _CACHE:
        _CACHE[key] = _build_program(with_qk_bias, with_v_bias)
    nc = _CACHE[key]

    # weight layout prep: wT[cin, ab*128 + c] = W[c, cin, a, b];
    # q and k weights packed into one tensor for a single early DMA
    wqT = Wq.transpose(1, 2, 3, 0).reshape(C, 4 * C)
    wkT = Wk.transpose(1, 2, 3, 0).reshape(C, 4 * C)
    wqk = np.ascontiguousarray(np.concatenate([wqT, wkT], axis=1))
    wv = np.ascontiguousarray(Wv.reshape(C, C))

    in_maps = []
    for b in range(B):
        m = {
            "x": np.ascontiguousarray(x[b].reshape(C, HW)),
            "wqk": wqk,
            "wv": wv,
        }
        if with_qk_bias:
            m["bq"] = np.ascontiguousarray(bq.reshape(C, 1))
            m["bk"] = np.ascontiguousarray(bk.reshape(C, 1))
        if with_v_bias:
            m["bv"] = np.ascontiguousarray(bv.reshape(C, 1))
        in_maps.append(m)

    res = run_bass_kernel_spmd(nc, in_maps, list(range(N_CORES)))
    out = np.stack([np.asarray(res.results[i]["out"], dtype=np.float32)
                    for i in range(N_CORES)])
    return out.reshape(B, C, H, W)
